# revision 39
# baseline (speedup 1.0000x reference)
# Trainium2 Bass kernel for nn_AttentionStream (dense transformer block with
# relative-position attention), SPMD over 8 NeuronCores.
#
# Sharding: core c -> batch b = c//2, head-group hg = c%2 (4 heads each).
# Each core computes a row-parallel partial of the output projection for its
# batch; the host sums the two partials per batch and adds the bias.
#
# v2 design ("fp8 DoubleRow" build):
#   - dots: q,k stored as e4m3 hi+lo residual pairs; one DoubleRow matmul
#     contracts all 4 cross terms (K=256 slots = 64 d x {hi,lo} x {dup}) at
#     0.5 cycles/col -> half the bf16 stream cost at >= bf16 precision.
#     Far-future clamp (d < -512) folds in via a third k block holding
#     k_lo + d_vec on the dup rows (selected by a strided block AP).
#   - pos[n, j] = q . relF[j]: fp8 DoubleRow (lhsT = q hi/lo, rhs = rel8
#     duplicated block); PSUM -> fp8 stage cast (vector/scalar split) ->
#     DRAM table with ODD row stride W=1281 so the skew diagonal gather is
#     expressible as a uint16 transpose-DMA (XBAR needs 2-byte dtype).  One
#     gather per rc-PAIR pulls 256 j-bytes/row = both rc's band tiles.
#   - band adds: DoubleRow matmul with an interleaved identity
#     W[p,i,m] = delta(m, 2*(p%64)+i) reading the u16-paired ep bytes.
#   - PV + outproj stay bf16 (fp8 P fails the 2e-2 gate); PV is interleaved
#     per-rc right behind exp so P needs only a 4-slot ring and the tail is
#     short.  exp applies the 1/8 softmax scale (q,k are stored unscaled).
# Engine budget: tensor ~273k cols-equiv; scalar = exp (+25% pos casts);
# vector = hi/lo builds + 75% pos casts + norm; gpsimd/sync = DMA triggers.
import os
import sys

import numpy as np
import ml_dtypes

for _p in ("/opt/trn_rl_repo", "/root/.axon_site/_ro/trn_rl_repo"):
    if _p not in sys.path and os.path.isdir(_p):
        sys.path.append(_p)

B, N, DIM = 4, 2048, 512
H, D = 8, 64          # total heads, head dim
HPC = 4               # heads per core
INNER = H * D
MAXP = 512
SCALE = D ** -0.5
NCORES = 8
W = 1281              # padded j width (ODD); j = PAD_L + 512 - d, d = n - r
WR = 1282             # rel8 SBUF inner stride (EVEN for PE ifmap APs)
PAD_L = 128
NRC = 16              # r-chunks of 128
NHALF = 2             # n-halves of 1024
HC = 1024             # half-window columns

BF = ml_dtypes.bfloat16
E4 = ml_dtypes.float8_e4m3   # TRN FP8_EXP4: max normal +-240

_CACHE = {}
K_NOPOS = os.environ.get("K_NOPOS", "0") == "1"
K_NOPV = os.environ.get("K_NOPV", "0") == "1"
K_NOGATHER = os.environ.get("K_NOGATHER", "0") == "1"
K_NOSTORE = os.environ.get("K_NOSTORE", "0") == "1"
K_NOBAND = os.environ.get("K_NOBAND", "0") == "1"


def _jrange(s):
    """Used (trimmed) j-range [jlo, jhi) of the pos table for n-half s."""
    if os.environ.get("K_NOTRIM", "0") == "1":
        return 0, W
    jlo = PAD_L + 512 + 128 * max(-4, -s) - 127
    jhi = PAD_L + 512 + 128 * min(4, 15 - s) + 127 + 1
    return max(0, jlo), min(W, jhi)


def _build_bass():
    import concourse.bass as bass
    import concourse.mybir as mybir
    import concourse.tile as tile
    from concourse import bacc

    dt = mybir.dt
    fp32 = dt.float32
    bf16 = dt.bfloat16
    fp8 = dt.float8e4
    u16 = dt.uint16
    EXP = mybir.ActivationFunctionType.Exp
    DR = mybir.MatmulPerfMode.DoubleRow

    nc = bacc.Bacc("TRN2", target_bir_lowering=False, debug=False,
                   num_devices=NCORES)

    xT = nc.dram_tensor("xT", [DIM, N], bf16, kind="ExternalInput")
    wq = nc.dram_tensor("wq", [DIM, 256], bf16, kind="ExternalInput")
    wk = nc.dram_tensor("wk", [DIM, 256], bf16, kind="ExternalInput")
    wv = nc.dram_tensor("wv", [DIM, 256], bf16, kind="ExternalInput")
    rel8 = nc.dram_tensor("rel8", [64, 2 * WR], fp8, kind="ExternalInput")
    wo = nc.dram_tensor("wo", [64, HPC * DIM], bf16, kind="ExternalInput")
    dv = nc.dram_tensor("dv", [128, 1], fp32, kind="ExternalInput")
    id8 = nc.dram_tensor("id8", [128, 256], fp8, kind="ExternalInput")
    outT = nc.dram_tensor("outT", [DIM, N], fp32, kind="ExternalOutput")

    from contextlib import ExitStack
    with tile.TileContext(nc) as tc, ExitStack() as ctx:
        consts = ctx.enter_context(tc.tile_pool(name="consts", bufs=1))
        ppool = ctx.enter_context(tc.tile_pool(name="ppool", bufs=4))
        stg = ctx.enter_context(tc.tile_pool(name="stg", bufs=6))
        eppool = ctx.enter_context(tc.tile_pool(name="eppool", bufs=3))
        rbpool = ctx.enter_context(tc.tile_pool(name="rbpool", bufs=2))
        pdots = ctx.enter_context(tc.tile_pool(name="pdots", bufs=2, space="PSUM"))
        ppos = ctx.enter_context(tc.tile_pool(name="ppos", bufs=2, space="PSUM"))
        pacc = ctx.enter_context(tc.tile_pool(name="pacc", bufs=1, space="PSUM"))
        dramp = ctx.enter_context(tc.tile_pool(name="dramp", bufs=4, space="DRAM"))

        # ---- load constants (spread queues; startup only) --------------------
        xT_sb = consts.tile([128, 4, N], bf16, tag="xT_sb")
        for dc, qeng in enumerate((nc.sync, nc.gpsimd, nc.sync, nc.gpsimd)):
            qeng.dma_start(out=xT_sb[:, dc, :],
                           in_=xT.ap()[dc * 128:(dc + 1) * 128, :])
        wq_sb = consts.tile([128, 4, 256], bf16, tag="wq_sb")
        nc.scalar.dma_start(out=wq_sb, in_=wq.ap().rearrange("(c p) i -> p c i", p=128))
        wk_sb = consts.tile([128, 4, 256], bf16, tag="wk_sb")
        nc.scalar.dma_start(out=wk_sb, in_=wk.ap().rearrange("(c p) i -> p c i", p=128))
        wv_sb = consts.tile([128, 4, 256], bf16, tag="wv_sb")
        nc.scalar.dma_start(out=wv_sb, in_=wv.ap().rearrange("(c p) i -> p c i", p=128))
        rel8_sb = consts.tile([64, 2, WR], fp8, tag="rel8_sb")
        nc.scalar.dma_start(out=rel8_sb, in_=rel8.ap().rearrange("p (i w) -> p i w", i=2))
        wo_sb = consts.tile([64, HPC, DIM], bf16, tag="wo_sb")
        nc.scalar.dma_start(out=wo_sb, in_=wo.ap().rearrange("p (h o) -> p h o", h=HPC))
        dv_sb = consts.tile([128, 1], fp32, tag="dv_sb")
        nc.scalar.dma_start(out=dv_sb, in_=dv.ap())
        id8_sb = consts.tile([128, 2, 128], fp8, tag="id8_sb")
        nc.scalar.dma_start(out=id8_sb, in_=id8.ap().rearrange("p (i m) -> p i m", i=2))

        # persistent per-head fp8 operands
        qq8 = [consts.tile([128, 2, N], fp8, tag=f"qq8_{h}", name=f"qq8_{h}")
               for h in range(HPC)]
        kk8 = [consts.tile([128, 3, N], fp8, tag=f"kk8_{h}", name=f"kk8_{h}")
               for h in range(HPC)]
        v_sb = consts.tile([128, NRC, HPC, 65], bf16, tag="v_sb")
        avn_all = consts.tile([64, HPC, NHALF, HC], bf16, tag="avn_all")

        # ---- projections -----------------------------------------------------
        def emit_proj_q(ic):
            """q proj for heads (2ic, 2ic+1); builds qq8 hi/lo + dup."""
            hA, hB = 2 * ic, 2 * ic + 1
            for nh in range(NHALF):
                ps = pdots.tile([128, HC], fp32, tag="dots", name="ps_projq")
                for sub in range(2):
                    for dcp in range(4):
                        nc.tensor.matmul(
                            ps[:, sub * 512:(sub + 1) * 512],
                            lhsT=wq_sb[:, dcp, ic * 128:(ic + 1) * 128],
                            rhs=xT_sb[:, dcp, nh * HC + sub * 512:
                                      nh * HC + (sub + 1) * 512],
                            start=(dcp == 0), stop=(dcp == 3))
                sl = slice(nh * HC, (nh + 1) * HC)
                # head A on ps[0:64]: (p<64) layout (i0=hi, i1=lo)
                nc.vector.tensor_copy(qq8[hA][0:64, 0, sl], ps[0:64, :])
                nc.vector.tensor_sub(qq8[hA][0:64, 1, sl], ps[0:64, :],
                                     qq8[hA][0:64, 0, sl])
                # head B on ps[64:128]: (p>=64) layout (i0=lo, i1=hi)
                nc.vector.tensor_copy(qq8[hB][64:128, 1, sl], ps[64:128, :])
                nc.vector.tensor_sub(qq8[hB][64:128, 0, sl], ps[64:128, :],
                                     qq8[hB][64:128, 1, sl])
                # dup with i-swap: (p<64)=(hi,lo) <-> (p>=64)=(lo,hi)
                nc.gpsimd.dma_start(out=qq8[hA][64:128, 1, sl], in_=qq8[hA][0:64, 0, sl])
                nc.gpsimd.dma_start(out=qq8[hA][64:128, 0, sl], in_=qq8[hA][0:64, 1, sl])
                nc.gpsimd.dma_start(out=qq8[hB][0:64, 0, sl], in_=qq8[hB][64:128, 1, sl])
                nc.gpsimd.dma_start(out=qq8[hB][0:64, 1, sl], in_=qq8[hB][64:128, 0, sl])

        def emit_proj_k(ic):
            """k proj; builds kk8 blocks (0=hi, 1=lo, 2=lo+dv) + dup."""
            hA, hB = 2 * ic, 2 * ic + 1
            for nh in range(NHALF):
                ps = pdots.tile([128, HC], fp32, tag="dots", name="ps_projk")
                for sub in range(2):
                    for dcp in range(4):
                        nc.tensor.matmul(
                            ps[:, sub * 512:(sub + 1) * 512],
                            lhsT=wk_sb[:, dcp, ic * 128:(ic + 1) * 128],
                            rhs=xT_sb[:, dcp, nh * HC + sub * 512:
                                      nh * HC + (sub + 1) * 512],
                            start=(dcp == 0), stop=(dcp == 3))
                sl = slice(nh * HC, (nh + 1) * HC)
                nc.vector.tensor_copy(kk8[hA][0:64, 0, sl], ps[0:64, :])
                nc.vector.tensor_sub(kk8[hA][0:64, 1, sl], ps[0:64, :],
                                     kk8[hA][0:64, 0, sl])
                nc.vector.tensor_copy(kk8[hB][64:128, 0, sl], ps[64:128, :])
                nc.vector.tensor_sub(kk8[hB][64:128, 1, sl], ps[64:128, :],
                                     kk8[hB][64:128, 0, sl])
                # dup (same order both halves)
                nc.sync.dma_start(out=kk8[hA][64:128, 0, sl], in_=kk8[hA][0:64, 0, sl])
                nc.sync.dma_start(out=kk8[hA][64:128, 1, sl], in_=kk8[hA][0:64, 1, sl])
                nc.sync.dma_start(out=kk8[hB][0:64, 0, sl], in_=kk8[hB][64:128, 0, sl])
                nc.sync.dma_start(out=kk8[hB][0:64, 1, sl], in_=kk8[hB][64:128, 1, sl])
                # block 2 = lo + dv (dv zero on p<64)
                for hh in (hA, hB):
                    nc.vector.tensor_scalar_add(kk8[hh][:, 2, sl],
                                                kk8[hh][:, 1, sl], dv_sb)

        def emit_proj_v():
            nc.vector.memset(v_sb[:, :, :, 64], 1.0)
            for rc in range(NRC):
                ps = pdots.tile([128, HC], fp32, tag="dots", name="ps_projv")
                for dcp in range(4):
                    nc.tensor.matmul(
                        ps[:, 0:256],
                        lhsT=xT_sb[:, dcp, rc * 128:(rc + 1) * 128],
                        rhs=wv_sb[:, dcp, :],
                        start=(dcp == 0), stop=(dcp == 3))
                nc.vector.tensor_copy(
                    v_sb[:, rc, :, 0:64],
                    ps[:, 0:256].rearrange("p (h d) -> p h d", h=HPC))

        # ---- pos tables ------------------------------------------------------
        pos_h = [None] * HPC
        pos_pieces = {}   # h -> list of (chunk, j0, jl)

        def plan_pos(h):
            pieces = []
            for s in range(16):
                jlo, jhi = _jrange(s)
                j0 = jlo
                while j0 < jhi:
                    jl = min(512, jhi - j0)
                    pieces.append((s, j0, jl))
                    j0 += jl
            pos_pieces[h] = pieces

        def emit_pos_piece(h, idx, ceng_sel=0, wide=False):
            """One pos piece: DoubleRow matmul -> fp8 cast -> DRAM store."""
            s, j0, jl = pos_pieces[h][idx]
            if pos_h[h] is None:
                pos_h[h] = dramp.tile([(N + 2) * W], fp8, tag="pos",
                                      name="pos_dram")
            pd = pos_h[h]
            if wide and idx % 2 == 0:
                pst = pdots.tile([128, HC], fp32, tag="dots", name="ps_pos2")
                ps = pst[:, 0:512]
            else:
                ps = ppos.tile([128, 512], fp32, tag="pos", name="ps_pos")
            nc.tensor.matmul(
                ps[:, 0:jl],
                lhsT=qq8[h][0:64, :, s * 128:(s + 1) * 128],
                rhs=rel8_sb[:, :, j0:j0 + jl],
                start=True, stop=True, perf_mode=DR)
            st = stg.tile([128, 512], fp8, tag="stage", name="stage")
            if ceng_sel == 0:
                nc.vector.tensor_copy(st[:, 0:jl], ps[:, 0:jl])
            else:
                nc.scalar.copy(st[:, 0:jl], ps[:, 0:jl])
            if not K_NOSTORE:
                dst = bass.AP(tensor=pd.tensor,
                              offset=pd.offset + s * 128 * W + j0,
                              ap=[[W, 128], [1, jl]])
                qeng = nc.gpsimd if (idx % 2 == 0) else nc.sync
                qeng.dma_start(out=dst, in_=st[:, 0:jl])

        def emit_pos(h, upfront=False):
            plan_pos(h)
            if K_NOPOS:
                return
            for idx in range(len(pos_pieces[h])):
                sel = (idx % 2) if upfront else (1 if idx % 4 == 3 else 0)
                emit_pos_piece(h, idx, sel)

        # ---- skew gathers (u16 transpose-DMA, one per rc pair) ---------------
        ep_h = [None] * HPC

        def emit_gather(h, geng=None):
            """ep[p, g, slot, 256B]: byte (2t+b) of slot s row c =
            pos[128 s + c, jb(2g, s) + 2t + b - c]."""
            if K_NOPOS or K_NOGATHER:
                return
            pd = pos_h[h]
            ep = eppool.tile([128, 8, 10, 256], fp8, tag="ep", name="ep_h")
            ep_h[h] = ep
            if os.environ.get("K_EPZERO", "0") == "1":
                nc.vector.memset(ep, 0.0)
            for g in range(8):
                rc = 2 * g
                s_lo = max(0, rc - 4)
                s_hi = min(15, rc + 5)
                k = s_hi - s_lo + 1
                off = (pd.offset + 128 * s_lo * W
                       + PAD_L + 512 + 128 * (rc - s_lo))
                src = bass.AP(tensor=pd.tensor, offset=off,
                              ap=[[W - 1, 128 * k], [1, 256]])
                dst = ep[:, g, 0:k, :]
                if geng == "split":
                    qeng = nc.sync if g % 2 == 0 else nc.scalar
                else:
                    qeng = geng or nc.sync
                qeng.dma_start(out=dst.bitcast(u16), in_=src.bitcast(u16),
                               transpose=True)

        # ---- dots + band + exp + PV per (h, half, rc) ------------------------
        P_rc = {}

        def emit_dots(h, half, rc):
            s0 = 8 * half
            ps = pdots.tile([128, HC], fp32, tag="dots", name="ps_dots")
            n0 = half * HC
            nd = (min(s0 + 7, rc - 5) - s0 + 1) * 128
            nd = max(0, min(nd, HC))
            kb = kk8[h]
            kba = kb[:, 0, rc * 128:(rc + 1) * 128]
            wl = bass.AP(tensor=kba.tensor, offset=kba.offset,
                         ap=[list(kba.ap[0]), [2 * N, 2], [1, 128]])
            wn = kb[:, 0:2, rc * 128:(rc + 1) * 128]
            cuts = sorted({0, HC, 512} | ({nd} if 0 < nd < HC else set()))
            for a, bnd in zip(cuts[:-1], cuts[1:]):
                nc.tensor.matmul(ps[:, a:bnd], lhsT=(wl if bnd <= nd else wn),
                                 rhs=qq8[h][:, :, n0 + a:n0 + bnd],
                                 start=True, stop=True, perf_mode=DR)
            # band add
            sa, sb = max(s0, rc - 4), min(s0 + 7, rc + 4)
            if (sb >= sa and not K_NOPOS and not K_NOGATHER and not K_NOBAND
                    and os.environ.get("K_BANDPAR", "") != str(1 - rc % 2)):
                cnt = sb - sa + 1
                c0 = (sa - s0) * 128
                g, e = rc // 2, rc % 2
                s_lo = max(0, 2 * g - 4)
                ep = ep_h[h]
                slot_sel = 0 if os.environ.get("K_BANDSLOT0", "0") == "1" else sa - s_lo
                epa = ep[64 * e:64 * (e + 1), g, slot_sel, 0:256]
                c1 = c0 + cnt * 128
                segs = [(a, bnd) for a, bnd in ((c0, min(c1, 512)), (max(c0, 512), c1))
                        if bnd > a]
                for a, bnd in segs:
                    xap = bass.AP(tensor=epa.tensor,
                                  offset=epa.offset + (a - c0) * 2,
                                  ap=[list(epa.ap[0]), [1, 2], [2, bnd - a]])
                    nc.tensor.matmul(ps[:, a:bnd],
                                     lhsT=id8_sb[64 * e:64 * (e + 1), :, :],
                                     rhs=xap,
                                     start=False, stop=True, perf_mode=DR,
                                     skip_group_check=True)
            P = ppool.tile([128, HC], bf16, tag="P", name="P_rc")
            P_rc[(h, half, rc)] = P
            nc.scalar.activation(P, ps, EXP, scale=0.125)

        acc_cur = [None]

        def emit_pv(h, half, rc):
            if rc == 0:
                acc_cur[0] = pacc.tile([128, HC], fp32, tag="acc", name="pv_acc")
            acc = acc_cur[0]
            P = P_rc.pop((h, half, rc))
            if K_NOPV:
                if rc == NRC - 1:
                    emit_norm(h, half, acc)
                return
            for sub in range(2):
                nc.tensor.matmul(acc[0:65, sub * 512:(sub + 1) * 512],
                                 lhsT=v_sb[:, rc, h, :],
                                 rhs=P[:, sub * 512:(sub + 1) * 512],
                                 start=(rc == 0), stop=(rc == NRC - 1),
                                 skip_group_check=True)
            if rc == NRC - 1:
                emit_norm(h, half, acc)

        recip_dram = {}

        def emit_norm(h, half, acc):
            # one PSUM->SBUF copy releases acc.  DVE op cost is free-size
            # bound, so the reciprocal runs on a [64, 16] reshape of the den
            # row (0.2us) instead of [*, 1024] (6.5us); DRAM hops do the
            # reshape and the 64-partition broadcast.
            num = rbpool.tile([65, HC], fp32, tag="num", name="num")
            nc.vector.tensor_copy(num, acc[0:65, :])
            rd = dramp.tile([1, HC], fp32, tag="recip", name="recip_dram")
            recip_dram[(h, half)] = rd
            nc.gpsimd.dma_start(out=rd, in_=num[64:65, :])
            rsq = rbpool.tile([64, 16], fp32, tag="rsq", name="rsq")
            rd2d = bass.AP(tensor=rd.tensor, offset=rd.offset,
                           ap=[[16, 64], [1, 16]])
            nc.gpsimd.dma_start(out=rsq, in_=rd2d)
            nc.vector.reciprocal(rsq, rsq)
            nc.gpsimd.dma_start(out=rd2d, in_=rsq)
            rb_bc = rbpool.tile([64, HC], fp32, tag="rb_bc", name="rb_bc")
            rsrc = bass.AP(tensor=rd.tensor, offset=rd.offset,
                           ap=[[0, 64], [1, HC]])
            nc.gpsimd.dma_start(out=rb_bc, in_=rsrc)
            nc.vector.tensor_mul(avn_all[:, h, half, :], num[0:64, :], rb_bc)

        def emit_outproj_unit(half, unit):
            # unit = oc*2 + sub; runs in a ppos [128, 512] tile so it never
            # contends with the PV accumulator
            oc, sub = unit // 2, unit % 2
            ps = ppos.tile([128, 512], fp32, tag="pos", name="ps_oproj")
            for h in range(HPC):
                nc.tensor.matmul(
                    ps,
                    lhsT=wo_sb[:, h, oc * 128:(oc + 1) * 128],
                    rhs=avn_all[:, h, half, sub * 512:(sub + 1) * 512],
                    start=(h == 0), stop=(h == HPC - 1))
            o_sb = rbpool.tile([128, 512], fp32, tag="o_sb", name="o_sb")
            nc.vector.tensor_copy(o_sb, ps)
            nc.sync.dma_start(
                out=outT.ap()[oc * 128:(oc + 1) * 128,
                              half * HC + sub * 512:half * HC + (sub + 1) * 512],
                in_=o_sb)

        def emit_outproj(half):
            for unit in range(8):
                emit_outproj_unit(half, unit)

        # ---- global schedule -------------------------------------------------
        # Startup: only pos(h0) runs upfront (scalar-biased casts while the
        # vector builds hi/lo); proj-v matmuls interleave between pos pieces
        # to keep the PE streaming while casts drain.  pos(h1) is paced into
        # h0's first half; pos(h2)/pos(h3) stream during h0/h1 as before.
        emit_proj_q(0)
        plan_pos(0)
        nc.vector.memset(v_sb[:, :, :, 64], 1.0)
        vrc = [0]

        def emit_projv_unit():
            if vrc[0] >= NRC:
                return
            rc = vrc[0]
            vrc[0] += 1
            ps = pdots.tile([128, HC], fp32, tag="dots", name="ps_projv")
            for dcp in range(4):
                nc.tensor.matmul(
                    ps[:, 0:256],
                    lhsT=xT_sb[:, dcp, rc * 128:(rc + 1) * 128],
                    rhs=wv_sb[:, dcp, :],
                    start=(dcp == 0), stop=(dcp == 3))
            nc.vector.tensor_copy(
                v_sb[:, rc, :, 0:64],
                ps[:, 0:256].rearrange("p (h d) -> p h d", h=HPC))

        emit_proj_q(1)
        emit_proj_k(0)
        emit_proj_k(1)
        if not K_NOPOS:
            for idx in range(len(pos_pieces[0])):
                emit_pos_piece(0, idx, idx % 2)
                if idx % 3 == 2:
                    emit_projv_unit()
        while vrc[0] < NRC:
            emit_projv_unit()
        emit_gather(0, geng="split")

        # head loop; the PV pipeline (prev) carries across (h, half)
        # boundaries so the tensor queue never drains on exp
        plan_pos(1)
        look = {0: 1, 1: 2, 2: 3, 3: None}   # h -> head whose pos streams now
        pace = {0: 16, 1: 32, 2: 32, 3: 1}   # slots to spread that head's pos over
        prev = []
        for h in range(HPC):
            hn = look[h]
            if hn is not None:
                plan_pos(hn)
            npieces = len(pos_pieces[hn]) if hn is not None else 0
            nslots = pace[h]
            pi = 0
            for half in range(NHALF):
                for rc in range(NRC):
                    emit_dots(h, half, rc)
                    if hn is not None and not K_NOPOS and rc >= 12:
                        slot = half * NRC + rc
                        target = -(-npieces * (slot + 1) // nslots)
                        while pi < min(target, npieces):
                            emit_pos_piece(hn, pi, 0)
                            pi += 1
                    prev.append((h, half, rc))
                    if len(prev) > 2:
                        emit_pv(*prev.pop(0))
                    if h == HPC - 1 and half == 1 and rc % 2 == 1 and rc // 2 < 8:
                        emit_outproj_unit(0, rc // 2)
                if h == HPC - 1:
                    while prev:
                        emit_pv(*prev.pop(0))
                    if half == 1:
                        emit_outproj(1)
                if h == 0 and half == 0:
                    emit_gather(1)
            if hn is not None and hn >= 2:
                emit_gather(hn)

    nc.compile()
    return nc


def host_prep(x, Wq, Wkv, Wo, bo, rel_emb):
    """Build the 8 per-core input maps (all host-side prep is O(N*D))."""
    x = np.asarray(x, np.float32)
    Wq = np.asarray(Wq, np.float32)
    Wkv = np.asarray(Wkv, np.float32)
    Wo = np.asarray(Wo, np.float32)
    rel_emb = np.asarray(rel_emb, np.float32)

    # relF[j] = rel_emb[1024-jgrid] - rel_emb[1024], edge-padded; [WR, 64]
    jgrid = np.clip(np.arange(WR) - PAD_L, 0, 1024)
    relF = rel_emb[1024 - jgrid] - rel_emb[1024]
    rel_one = np.ascontiguousarray(relF.T)             # [64, WR]
    rel8_in = np.concatenate([rel_one, rel_one], axis=1)  # [64, 2WR] dup blocks
    rel8_in = np.clip(rel8_in, -240, 240).astype(E4)
    d_vec = rel_emb[0] - rel_emb[1024]                 # [64] far-future clamp
    dv_in = np.concatenate([np.zeros(64, np.float32), d_vec]).reshape(128, 1)
    dv_in = dv_in.astype(np.float32)
    # interleaved identity: W[p, i, m] = delta(m, 2*(p%64)+i)
    id_in = np.zeros((128, 2, 128), np.float32)
    for p in range(128):
        for i in range(2):
            id_in[p, i, 2 * (p % 64) + i] = 1.0
    id8_in = id_in.reshape(128, 256).astype(E4)

    in_maps = []
    for core in range(NCORES):
        b, hg = core // 2, core % 2
        sl = slice(hg * 256, (hg + 1) * 256)
        in_maps.append({
            "xT": np.ascontiguousarray(x[b].T).astype(BF),
            "wq": Wq[:, sl].astype(BF),               # NOTE: unscaled; exp applies 1/8
            "wk": Wkv[:, sl].astype(BF),
            "wv": Wkv[:, 512 + hg * 256: 512 + (hg + 1) * 256].astype(BF),
            "rel8": rel8_in,
            "wo": np.ascontiguousarray(Wo[sl, :].reshape(HPC, 64, DIM)
                                       .transpose(1, 0, 2).reshape(64, HPC * DIM)),
            "dv": dv_in,
            "id8": id8_in,
        })
        in_maps[-1]["wo"] = in_maps[-1]["wo"].astype(BF)
    return in_maps


def _install_ntff_hook():
    """The agent image's antenv lacks axon_hooks; synthesize it so
    run_bass_kernel_spmd(trace=True) can capture NTFF profiles."""
    import types
    try:
        if "antenv.axon_hooks" not in sys.modules:
            import antenv
            from trn_agent_boot.trn_boot import _ntff_profile_via_ctypes
            hooks = types.ModuleType("antenv.axon_hooks")
            state = {"h": _ntff_profile_via_ctypes("/opt/axon/libaxon_pjrt.so")}
            hooks.set_axon_ntff_profile_hook = lambda h: state.__setitem__("h", h)
            hooks.get_axon_ntff_profile_hook = lambda: state["h"]
            sys.modules["antenv.axon_hooks"] = hooks
            antenv.axon_hooks = hooks
        import antenv.axon_hooks as ah
        return ah.get_axon_ntff_profile_hook() is not None
    except Exception as e:
        print(f"ntff hook install failed: {e!r}")
        return False


def kernel(x, Wq, Wkv, Wo, bo, rel_emb, _trace=False):
    import concourse.bass_utils as bu
    from concourse.bass_utils import run_bass_kernel_spmd

    if "nc" not in _CACHE:
        _CACHE["nc"] = _build_bass()
    nc = _CACHE["nc"]

    in_maps = host_prep(x, Wq, Wkv, Wo, bo, rel_emb)
    kw = {}
    if _trace and _install_ntff_hook():
        bu.upload_artifacts = lambda d: d     # zero-egress: keep artifacts local
        tmpdir = "/root/problem/traces/latest"
        import shutil
        shutil.rmtree(tmpdir, ignore_errors=True)
        os.makedirs(tmpdir, exist_ok=True)
        kw = dict(trace=True, tmpdir=tmpdir)
    res = run_bass_kernel_spmd(nc, in_maps, list(range(NCORES)), **kw)
    _CACHE["last_result"] = res

    bo = np.asarray(bo, np.float32)
    out = np.empty((B, N, DIM), np.float32)
    for b in range(B):
        pT = res.results[2 * b]["outT"] + res.results[2 * b + 1]["outT"]
        out[b] = pT.T + bo[None, :]
    return out


# revision 40
# speedup vs baseline: 1.1760x; 1.1760x over previous
# Trainium2 Bass kernel for nn_AttentionStream (dense transformer block with
# relative-position attention), SPMD over 8 NeuronCores.
#
# Sharding: core c -> batch b = c//2, head-group hg = c%2 (4 heads each).
# Each core computes a row-parallel partial of the output projection for its
# batch; the host sums the two partials per batch and adds the bias.
#
# v2 design ("fp8 DoubleRow" build):
#   - dots: q,k stored as e4m3 hi+lo residual pairs; one DoubleRow matmul
#     contracts all 4 cross terms (K=256 slots = 64 d x {hi,lo} x {dup}) at
#     0.5 cycles/col -> half the bf16 stream cost at >= bf16 precision.
#     Far-future clamp (d < -512) folds in via a third k block holding
#     k_lo + d_vec on the dup rows (selected by a strided block AP).
#   - pos[n, j] = q . relF[j]: fp8 DoubleRow (lhsT = q hi/lo, rhs = rel8
#     duplicated block); PSUM -> fp8 stage cast (vector/scalar split) ->
#     DRAM table with ODD row stride W=1281 so the skew diagonal gather is
#     expressible as a uint16 transpose-DMA (XBAR needs 2-byte dtype).  One
#     gather per rc-PAIR pulls 256 j-bytes/row = both rc's band tiles.
#   - band adds: DoubleRow matmul with an interleaved identity
#     W[p,i,m] = delta(m, 2*(p%64)+i) reading the u16-paired ep bytes.
#   - PV + outproj stay bf16 (fp8 P fails the 2e-2 gate); PV is interleaved
#     per-rc right behind exp so P needs only a 4-slot ring and the tail is
#     short.  exp applies the 1/8 softmax scale (q,k are stored unscaled).
# Engine budget: tensor ~273k cols-equiv; scalar = exp (+25% pos casts);
# vector = hi/lo builds + 75% pos casts + norm; gpsimd/sync = DMA triggers.
import os
import sys

import numpy as np
import ml_dtypes

for _p in ("/opt/trn_rl_repo", "/root/.axon_site/_ro/trn_rl_repo"):
    if _p not in sys.path and os.path.isdir(_p):
        sys.path.append(_p)

B, N, DIM = 4, 2048, 512
H, D = 8, 64          # total heads, head dim
HPC = 4               # heads per core
INNER = H * D
MAXP = 512
SCALE = D ** -0.5
NCORES = 8
W = 1281              # padded j width (ODD); j = PAD_L + 512 - d, d = n - r
WR = 1282             # rel8 SBUF inner stride (EVEN for PE ifmap APs)
PAD_L = 128
NRC = 16              # r-chunks of 128
NHALF = 2             # n-halves of 1024
HC = 1024             # half-window columns

BF = ml_dtypes.bfloat16
E4 = ml_dtypes.float8_e4m3   # TRN FP8_EXP4: max normal +-240

_CACHE = {}
K_NOPOS = os.environ.get("K_NOPOS", "0") == "1"
K_NOPV = os.environ.get("K_NOPV", "0") == "1"
K_NOGATHER = os.environ.get("K_NOGATHER", "0") == "1"
K_NOSTORE = os.environ.get("K_NOSTORE", "0") == "1"
K_NOBAND = os.environ.get("K_NOBAND", "0") == "1"


def _jrange(s):
    """Used (trimmed) j-range [jlo, jhi) of the pos table for n-half s."""
    if os.environ.get("K_NOTRIM", "0") == "1":
        return 0, W
    jlo = PAD_L + 512 + 128 * max(-4, -s) - 127
    jhi = PAD_L + 512 + 128 * min(4, 15 - s) + 127 + 1
    return max(0, jlo), min(W, jhi)


def _build_bass():
    import concourse.bass as bass
    import concourse.mybir as mybir
    import concourse.tile as tile
    from concourse import bacc

    dt = mybir.dt
    fp32 = dt.float32
    bf16 = dt.bfloat16
    fp8 = dt.float8e4
    u16 = dt.uint16
    EXP = mybir.ActivationFunctionType.Exp
    DR = mybir.MatmulPerfMode.DoubleRow

    nc = bacc.Bacc("TRN2", target_bir_lowering=False, debug=False,
                   num_devices=NCORES)

    xT = nc.dram_tensor("xT", [DIM, N], bf16, kind="ExternalInput")
    wq = nc.dram_tensor("wq", [DIM, 256], bf16, kind="ExternalInput")
    wk = nc.dram_tensor("wk", [DIM, 256], bf16, kind="ExternalInput")
    wv = nc.dram_tensor("wv", [DIM, 256], bf16, kind="ExternalInput")
    rel8 = nc.dram_tensor("rel8", [64, 2 * WR], fp8, kind="ExternalInput")
    wo = nc.dram_tensor("wo", [64, HPC * DIM], bf16, kind="ExternalInput")
    dv = nc.dram_tensor("dv", [128, 1], fp32, kind="ExternalInput")
    id8 = nc.dram_tensor("id8", [128, 256], fp8, kind="ExternalInput")
    outT = nc.dram_tensor("outT", [DIM, N], fp32, kind="ExternalOutput")

    from contextlib import ExitStack
    with tile.TileContext(nc) as tc, ExitStack() as ctx:
        consts = ctx.enter_context(tc.tile_pool(name="consts", bufs=1))
        ppool = ctx.enter_context(tc.tile_pool(name="ppool", bufs=4))
        stg = ctx.enter_context(tc.tile_pool(name="stg", bufs=6))
        eppool = ctx.enter_context(tc.tile_pool(name="eppool", bufs=3))
        rbpool = ctx.enter_context(tc.tile_pool(name="rbpool", bufs=2))
        pdots = ctx.enter_context(tc.tile_pool(name="pdots", bufs=2, space="PSUM"))
        ppos = ctx.enter_context(tc.tile_pool(name="ppos", bufs=2, space="PSUM"))
        pacc = ctx.enter_context(tc.tile_pool(name="pacc", bufs=1, space="PSUM"))
        dramp = ctx.enter_context(tc.tile_pool(name="dramp", bufs=4, space="DRAM"))

        # ---- load constants (spread queues; startup only) --------------------
        xT_sb = consts.tile([128, 4, N], bf16, tag="xT_sb")
        for dc, qeng in enumerate((nc.sync, nc.scalar, nc.sync, nc.scalar)):
            qeng.dma_start(out=xT_sb[:, dc, :],
                           in_=xT.ap()[dc * 128:(dc + 1) * 128, :])
        wq_sb = consts.tile([128, 4, 256], bf16, tag="wq_sb")
        nc.scalar.dma_start(out=wq_sb, in_=wq.ap().rearrange("(c p) i -> p c i", p=128))
        wk_sb = consts.tile([128, 4, 256], bf16, tag="wk_sb")
        nc.scalar.dma_start(out=wk_sb, in_=wk.ap().rearrange("(c p) i -> p c i", p=128))
        wv_sb = consts.tile([128, 4, 256], bf16, tag="wv_sb")
        nc.scalar.dma_start(out=wv_sb, in_=wv.ap().rearrange("(c p) i -> p c i", p=128))
        rel8_sb = consts.tile([64, 2, WR], fp8, tag="rel8_sb")
        nc.scalar.dma_start(out=rel8_sb, in_=rel8.ap().rearrange("p (i w) -> p i w", i=2))
        wo_sb = consts.tile([64, HPC, DIM], bf16, tag="wo_sb")
        nc.scalar.dma_start(out=wo_sb, in_=wo.ap().rearrange("p (h o) -> p h o", h=HPC))
        dv_sb = consts.tile([128, 1], fp32, tag="dv_sb")
        nc.scalar.dma_start(out=dv_sb, in_=dv.ap())
        id8_sb = consts.tile([128, 2, 128], fp8, tag="id8_sb")
        nc.scalar.dma_start(out=id8_sb, in_=id8.ap().rearrange("p (i m) -> p i m", i=2))

        # persistent per-head fp8 operands
        qq8 = [consts.tile([128, 2, N], fp8, tag=f"qq8_{h}", name=f"qq8_{h}")
               for h in range(HPC)]
        kk8 = [consts.tile([128, 3, N], fp8, tag=f"kk8_{h}", name=f"kk8_{h}")
               for h in range(HPC)]
        v_sb = consts.tile([128, NRC, HPC, 65], bf16, tag="v_sb")
        avn_all = consts.tile([64, HPC, NHALF, HC], bf16, tag="avn_all")

        # ---- projections -----------------------------------------------------
        def emit_proj_q(ic):
            """q proj for heads (2ic, 2ic+1); builds qq8 hi/lo + dup."""
            hA, hB = 2 * ic, 2 * ic + 1
            for nh in range(NHALF):
                ps = pdots.tile([128, HC], fp32, tag="dots", name="ps_projq")
                for sub in range(2):
                    for dcp in range(4):
                        nc.tensor.matmul(
                            ps[:, sub * 512:(sub + 1) * 512],
                            lhsT=wq_sb[:, dcp, ic * 128:(ic + 1) * 128],
                            rhs=xT_sb[:, dcp, nh * HC + sub * 512:
                                      nh * HC + (sub + 1) * 512],
                            start=(dcp == 0), stop=(dcp == 3))
                sl = slice(nh * HC, (nh + 1) * HC)
                # head A on ps[0:64]: (p<64) layout (i0=hi, i1=lo)
                nc.vector.tensor_copy(qq8[hA][0:64, 0, sl], ps[0:64, :])
                nc.vector.tensor_sub(qq8[hA][0:64, 1, sl], ps[0:64, :],
                                     qq8[hA][0:64, 0, sl])
                # head B on ps[64:128]: (p>=64) layout (i0=lo, i1=hi)
                nc.vector.tensor_copy(qq8[hB][64:128, 1, sl], ps[64:128, :])
                nc.vector.tensor_sub(qq8[hB][64:128, 0, sl], ps[64:128, :],
                                     qq8[hB][64:128, 1, sl])
                # dup with i-swap: (p<64)=(hi,lo) <-> (p>=64)=(lo,hi)
                nc.gpsimd.dma_start(out=qq8[hA][64:128, 1, sl], in_=qq8[hA][0:64, 0, sl])
                nc.gpsimd.dma_start(out=qq8[hA][64:128, 0, sl], in_=qq8[hA][0:64, 1, sl])
                nc.gpsimd.dma_start(out=qq8[hB][0:64, 0, sl], in_=qq8[hB][64:128, 1, sl])
                nc.gpsimd.dma_start(out=qq8[hB][0:64, 1, sl], in_=qq8[hB][64:128, 0, sl])

        def emit_proj_k(ic):
            """k proj; builds kk8 blocks (0=hi, 1=lo, 2=lo+dv) + dup."""
            hA, hB = 2 * ic, 2 * ic + 1
            for nh in range(NHALF):
                ps = pdots.tile([128, HC], fp32, tag="dots", name="ps_projk")
                for sub in range(2):
                    for dcp in range(4):
                        nc.tensor.matmul(
                            ps[:, sub * 512:(sub + 1) * 512],
                            lhsT=wk_sb[:, dcp, ic * 128:(ic + 1) * 128],
                            rhs=xT_sb[:, dcp, nh * HC + sub * 512:
                                      nh * HC + (sub + 1) * 512],
                            start=(dcp == 0), stop=(dcp == 3))
                sl = slice(nh * HC, (nh + 1) * HC)
                nc.vector.tensor_copy(kk8[hA][0:64, 0, sl], ps[0:64, :])
                nc.vector.tensor_sub(kk8[hA][0:64, 1, sl], ps[0:64, :],
                                     kk8[hA][0:64, 0, sl])
                nc.vector.tensor_copy(kk8[hB][64:128, 0, sl], ps[64:128, :])
                nc.vector.tensor_sub(kk8[hB][64:128, 1, sl], ps[64:128, :],
                                     kk8[hB][64:128, 0, sl])
                # dup (same order both halves)
                nc.sync.dma_start(out=kk8[hA][64:128, 0, sl], in_=kk8[hA][0:64, 0, sl])
                nc.sync.dma_start(out=kk8[hA][64:128, 1, sl], in_=kk8[hA][0:64, 1, sl])
                nc.sync.dma_start(out=kk8[hB][0:64, 0, sl], in_=kk8[hB][64:128, 0, sl])
                nc.sync.dma_start(out=kk8[hB][0:64, 1, sl], in_=kk8[hB][64:128, 1, sl])
                # block 2 = lo + dv (dv zero on p<64)
                for hh in (hA, hB):
                    nc.vector.tensor_scalar_add(kk8[hh][:, 2, sl],
                                                kk8[hh][:, 1, sl], dv_sb)

        def emit_proj_v():
            nc.vector.memset(v_sb[:, :, :, 64], 1.0)
            for rc in range(NRC):
                ps = pdots.tile([128, HC], fp32, tag="dots", name="ps_projv")
                for dcp in range(4):
                    nc.tensor.matmul(
                        ps[:, 0:256],
                        lhsT=xT_sb[:, dcp, rc * 128:(rc + 1) * 128],
                        rhs=wv_sb[:, dcp, :],
                        start=(dcp == 0), stop=(dcp == 3))
                nc.vector.tensor_copy(
                    v_sb[:, rc, :, 0:64],
                    ps[:, 0:256].rearrange("p (h d) -> p h d", h=HPC))

        # ---- pos tables ------------------------------------------------------
        pos_h = [None] * HPC
        pos_pieces = {}   # h -> list of (chunk, j0, jl)

        def plan_pos(h):
            pieces = []
            for s in range(16):
                jlo, jhi = _jrange(s)
                j0 = jlo
                while j0 < jhi:
                    jl = min(512, jhi - j0)
                    pieces.append((s, j0, jl))
                    j0 += jl
            pos_pieces[h] = pieces

        def emit_pos_piece(h, idx, ceng_sel=0, wide=False):
            """One pos piece: DoubleRow matmul -> fp8 cast -> DRAM store."""
            s, j0, jl = pos_pieces[h][idx]
            if pos_h[h] is None:
                pos_h[h] = dramp.tile([(N + 2) * W], fp8, tag="pos",
                                      name="pos_dram")
            pd = pos_h[h]
            if wide and idx % 2 == 0:
                pst = pdots.tile([128, HC], fp32, tag="dots", name="ps_pos2")
                ps = pst[:, 0:512]
            else:
                ps = ppos.tile([128, 512], fp32, tag="pos", name="ps_pos")
            nc.tensor.matmul(
                ps[:, 0:jl],
                lhsT=qq8[h][0:64, :, s * 128:(s + 1) * 128],
                rhs=rel8_sb[:, :, j0:j0 + jl],
                start=True, stop=True, perf_mode=DR)
            st = stg.tile([128, 512], fp8, tag="stage", name="stage")
            if ceng_sel == 0:
                nc.vector.tensor_copy(st[:, 0:jl], ps[:, 0:jl])
            else:
                nc.scalar.copy(st[:, 0:jl], ps[:, 0:jl])
            if not K_NOSTORE:
                dst = bass.AP(tensor=pd.tensor,
                              offset=pd.offset + s * 128 * W + j0,
                              ap=[[W, 128], [1, jl]])
                qeng = nc.gpsimd if (idx % 2 == 0) else nc.sync
                qeng.dma_start(out=dst, in_=st[:, 0:jl])

        def emit_pos(h, upfront=False):
            plan_pos(h)
            if K_NOPOS:
                return
            for idx in range(len(pos_pieces[h])):
                sel = (idx % 2) if upfront else (1 if idx % 4 == 3 else 0)
                emit_pos_piece(h, idx, sel)

        # ---- skew gathers (u16 transpose-DMA, one per rc pair) ---------------
        ep_h = [None] * HPC

        def emit_gather(h, geng=None):
            """ep[p, g, slot, 256B]: byte (2t+b) of slot s row c =
            pos[128 s + c, jb(2g, s) + 2t + b - c]."""
            if K_NOPOS or K_NOGATHER:
                return
            pd = pos_h[h]
            ep = eppool.tile([128, 8, 10, 256], fp8, tag="ep", name="ep_h")
            ep_h[h] = ep
            if os.environ.get("K_EPZERO", "0") == "1":
                nc.vector.memset(ep, 0.0)
            for g in range(8):
                rc = 2 * g
                s_lo = max(0, rc - 4)
                s_hi = min(15, rc + 5)
                k = s_hi - s_lo + 1
                off = (pd.offset + 128 * s_lo * W
                       + PAD_L + 512 + 128 * (rc - s_lo))
                src = bass.AP(tensor=pd.tensor, offset=off,
                              ap=[[W - 1, 128 * k], [1, 256]])
                dst = ep[:, g, 0:k, :]
                if geng == "split":
                    qeng = nc.sync if g % 2 == 0 else nc.scalar
                else:
                    qeng = geng or nc.sync
                qeng.dma_start(out=dst.bitcast(u16), in_=src.bitcast(u16),
                               transpose=True)

        # ---- dots + band + exp + PV per (h, half, rc) ------------------------
        P_rc = {}

        def emit_dots(h, half, rc):
            s0 = 8 * half
            ps = pdots.tile([128, HC], fp32, tag="dots", name="ps_dots")
            n0 = half * HC
            nd = (min(s0 + 7, rc - 5) - s0 + 1) * 128
            nd = max(0, min(nd, HC))
            kb = kk8[h]
            kba = kb[:, 0, rc * 128:(rc + 1) * 128]
            wl = bass.AP(tensor=kba.tensor, offset=kba.offset,
                         ap=[list(kba.ap[0]), [2 * N, 2], [1, 128]])
            wn = kb[:, 0:2, rc * 128:(rc + 1) * 128]
            cuts = sorted({0, HC, 512} | ({nd} if 0 < nd < HC else set()))
            for a, bnd in zip(cuts[:-1], cuts[1:]):
                nc.tensor.matmul(ps[:, a:bnd], lhsT=(wl if bnd <= nd else wn),
                                 rhs=qq8[h][:, :, n0 + a:n0 + bnd],
                                 start=True, stop=True, perf_mode=DR)
            # band add
            sa, sb = max(s0, rc - 4), min(s0 + 7, rc + 4)
            if (sb >= sa and not K_NOPOS and not K_NOGATHER and not K_NOBAND
                    and os.environ.get("K_BANDPAR", "") != str(1 - rc % 2)):
                cnt = sb - sa + 1
                c0 = (sa - s0) * 128
                g, e = rc // 2, rc % 2
                s_lo = max(0, 2 * g - 4)
                ep = ep_h[h]
                slot_sel = 0 if os.environ.get("K_BANDSLOT0", "0") == "1" else sa - s_lo
                epa = ep[64 * e:64 * (e + 1), g, slot_sel, 0:256]
                c1 = c0 + cnt * 128
                segs = [(a, bnd) for a, bnd in ((c0, min(c1, 512)), (max(c0, 512), c1))
                        if bnd > a]
                for a, bnd in segs:
                    xap = bass.AP(tensor=epa.tensor,
                                  offset=epa.offset + (a - c0) * 2,
                                  ap=[list(epa.ap[0]), [1, 2], [2, bnd - a]])
                    nc.tensor.matmul(ps[:, a:bnd],
                                     lhsT=id8_sb[64 * e:64 * (e + 1), :, :],
                                     rhs=xap,
                                     start=False, stop=True, perf_mode=DR,
                                     skip_group_check=True)
            P = ppool.tile([128, HC], bf16, tag="P", name="P_rc")
            P_rc[(h, half, rc)] = P
            nc.scalar.activation(P, ps, EXP, scale=0.125)

        acc_cur = [None]

        def emit_pv(h, half, rc):
            if rc == 0:
                acc_cur[0] = pacc.tile([128, HC], fp32, tag="acc", name="pv_acc")
            acc = acc_cur[0]
            P = P_rc.pop((h, half, rc))
            if K_NOPV:
                if rc == NRC - 1:
                    emit_norm(h, half, acc)
                return
            for sub in range(2):
                nc.tensor.matmul(acc[0:65, sub * 512:(sub + 1) * 512],
                                 lhsT=v_sb[:, rc, h, :],
                                 rhs=P[:, sub * 512:(sub + 1) * 512],
                                 start=(rc == 0), stop=(rc == NRC - 1),
                                 skip_group_check=True)
            if rc == NRC - 1:
                emit_norm(h, half, acc)

        recip_dram = {}

        def emit_norm(h, half, acc):
            # one PSUM->SBUF copy releases acc.  DVE op cost is free-size
            # bound, so the reciprocal runs on a [64, 16] reshape of the den
            # row (0.2us) instead of [*, 1024] (6.5us); DRAM hops do the
            # reshape and the 64-partition broadcast.
            num = rbpool.tile([65, HC], fp32, tag="num", name="num")
            nc.vector.tensor_copy(num, acc[0:65, :])
            rd = dramp.tile([1, HC], fp32, tag="recip", name="recip_dram")
            recip_dram[(h, half)] = rd
            nc.gpsimd.dma_start(out=rd, in_=num[64:65, :])
            rsq = rbpool.tile([64, 16], fp32, tag="rsq", name="rsq")
            rd2d = bass.AP(tensor=rd.tensor, offset=rd.offset,
                           ap=[[16, 64], [1, 16]])
            nc.gpsimd.dma_start(out=rsq, in_=rd2d)
            nc.vector.reciprocal(rsq, rsq)
            nc.gpsimd.dma_start(out=rd2d, in_=rsq)
            rb_bc = rbpool.tile([64, HC], fp32, tag="rb_bc", name="rb_bc")
            rsrc = bass.AP(tensor=rd.tensor, offset=rd.offset,
                           ap=[[0, 64], [1, HC]])
            nc.gpsimd.dma_start(out=rb_bc, in_=rsrc)
            nc.vector.tensor_mul(avn_all[:, h, half, :], num[0:64, :], rb_bc)

        def emit_outproj_unit(half, unit):
            # unit = oc*2 + sub; runs in a ppos [128, 512] tile so it never
            # contends with the PV accumulator
            oc, sub = unit // 2, unit % 2
            ps = ppos.tile([128, 512], fp32, tag="pos", name="ps_oproj")
            for h in range(HPC):
                nc.tensor.matmul(
                    ps,
                    lhsT=wo_sb[:, h, oc * 128:(oc + 1) * 128],
                    rhs=avn_all[:, h, half, sub * 512:(sub + 1) * 512],
                    start=(h == 0), stop=(h == HPC - 1))
            o_sb = rbpool.tile([128, 512], fp32, tag="o_sb", name="o_sb")
            nc.vector.tensor_copy(o_sb, ps)
            nc.sync.dma_start(
                out=outT.ap()[oc * 128:(oc + 1) * 128,
                              half * HC + sub * 512:half * HC + (sub + 1) * 512],
                in_=o_sb)

        def emit_outproj(half):
            for unit in range(8):
                emit_outproj_unit(half, unit)

        # ---- global schedule -------------------------------------------------
        # Startup: only pos(h0) runs upfront (scalar-biased casts while the
        # vector builds hi/lo); proj-v matmuls interleave between pos pieces
        # to keep the PE streaming while casts drain.  pos(h1) is paced into
        # h0's first half; pos(h2)/pos(h3) stream during h0/h1 as before.
        emit_proj_q(0)
        plan_pos(0)
        nc.vector.memset(v_sb[:, :, :, 64], 1.0)
        vrc = [0]

        def emit_projv_unit():
            if vrc[0] >= NRC:
                return
            rc = vrc[0]
            vrc[0] += 1
            ps = pdots.tile([128, HC], fp32, tag="dots", name="ps_projv")
            for dcp in range(4):
                nc.tensor.matmul(
                    ps[:, 0:256],
                    lhsT=xT_sb[:, dcp, rc * 128:(rc + 1) * 128],
                    rhs=wv_sb[:, dcp, :],
                    start=(dcp == 0), stop=(dcp == 3))
            nc.vector.tensor_copy(
                v_sb[:, rc, :, 0:64],
                ps[:, 0:256].rearrange("p (h d) -> p h d", h=HPC))

        emit_proj_q(1)
        emit_proj_k(0)
        emit_proj_k(1)
        if not K_NOPOS:
            for idx in range(len(pos_pieces[0])):
                emit_pos_piece(0, idx, idx % 2)
                if idx % 3 == 2:
                    emit_projv_unit()
        while vrc[0] < NRC:
            emit_projv_unit()
        emit_gather(0, geng="split")

        # head loop; the PV pipeline (prev) carries across (h, half)
        # boundaries so the tensor queue never drains on exp
        plan_pos(1)
        look = {0: 1, 1: 2, 2: 3, 3: None}   # h -> head whose pos streams now
        pace = {0: 16, 1: 32, 2: 32, 3: 1}   # slots to spread that head's pos over
        prev = []
        for h in range(HPC):
            hn = look[h]
            if hn is not None:
                plan_pos(hn)
            npieces = len(pos_pieces[hn]) if hn is not None else 0
            nslots = pace[h]
            pi = 0
            for half in range(NHALF):
                for rc in range(NRC):
                    emit_dots(h, half, rc)
                    if hn is not None and not K_NOPOS and rc >= 12:
                        slot = half * NRC + rc
                        target = -(-npieces * (slot + 1) // nslots)
                        while pi < min(target, npieces):
                            emit_pos_piece(hn, pi, 0)
                            pi += 1
                    prev.append((h, half, rc))
                    if len(prev) > 2:
                        emit_pv(*prev.pop(0))
                    if h == HPC - 1 and half == 1 and rc % 2 == 1 and rc // 2 < 8:
                        emit_outproj_unit(0, rc // 2)
                if h == HPC - 1:
                    while prev:
                        emit_pv(*prev.pop(0))
                    if half == 1:
                        emit_outproj(1)
                if h == 0 and half == 0:
                    emit_gather(1)
            if hn is not None and hn >= 2:
                emit_gather(hn)

    nc.compile()
    return nc


def host_prep(x, Wq, Wkv, Wo, bo, rel_emb):
    """Build the 8 per-core input maps (all host-side prep is O(N*D))."""
    x = np.asarray(x, np.float32)
    Wq = np.asarray(Wq, np.float32)
    Wkv = np.asarray(Wkv, np.float32)
    Wo = np.asarray(Wo, np.float32)
    rel_emb = np.asarray(rel_emb, np.float32)

    # relF[j] = rel_emb[1024-jgrid] - rel_emb[1024], edge-padded; [WR, 64]
    jgrid = np.clip(np.arange(WR) - PAD_L, 0, 1024)
    relF = rel_emb[1024 - jgrid] - rel_emb[1024]
    rel_one = np.ascontiguousarray(relF.T)             # [64, WR]
    rel8_in = np.concatenate([rel_one, rel_one], axis=1)  # [64, 2WR] dup blocks
    rel8_in = np.clip(rel8_in, -240, 240).astype(E4)
    d_vec = rel_emb[0] - rel_emb[1024]                 # [64] far-future clamp
    dv_in = np.concatenate([np.zeros(64, np.float32), d_vec]).reshape(128, 1)
    dv_in = dv_in.astype(np.float32)
    # interleaved identity: W[p, i, m] = delta(m, 2*(p%64)+i)
    id_in = np.zeros((128, 2, 128), np.float32)
    for p in range(128):
        for i in range(2):
            id_in[p, i, 2 * (p % 64) + i] = 1.0
    id8_in = id_in.reshape(128, 256).astype(E4)

    in_maps = []
    for core in range(NCORES):
        b, hg = core // 2, core % 2
        sl = slice(hg * 256, (hg + 1) * 256)
        in_maps.append({
            "xT": np.ascontiguousarray(x[b].T).astype(BF),
            "wq": Wq[:, sl].astype(BF),               # NOTE: unscaled; exp applies 1/8
            "wk": Wkv[:, sl].astype(BF),
            "wv": Wkv[:, 512 + hg * 256: 512 + (hg + 1) * 256].astype(BF),
            "rel8": rel8_in,
            "wo": np.ascontiguousarray(Wo[sl, :].reshape(HPC, 64, DIM)
                                       .transpose(1, 0, 2).reshape(64, HPC * DIM)),
            "dv": dv_in,
            "id8": id8_in,
        })
        in_maps[-1]["wo"] = in_maps[-1]["wo"].astype(BF)
    return in_maps


def _install_ntff_hook():
    """The agent image's antenv lacks axon_hooks; synthesize it so
    run_bass_kernel_spmd(trace=True) can capture NTFF profiles."""
    import types
    try:
        if "antenv.axon_hooks" not in sys.modules:
            import antenv
            from trn_agent_boot.trn_boot import _ntff_profile_via_ctypes
            hooks = types.ModuleType("antenv.axon_hooks")
            state = {"h": _ntff_profile_via_ctypes("/opt/axon/libaxon_pjrt.so")}
            hooks.set_axon_ntff_profile_hook = lambda h: state.__setitem__("h", h)
            hooks.get_axon_ntff_profile_hook = lambda: state["h"]
            sys.modules["antenv.axon_hooks"] = hooks
            antenv.axon_hooks = hooks
        import antenv.axon_hooks as ah
        return ah.get_axon_ntff_profile_hook() is not None
    except Exception as e:
        print(f"ntff hook install failed: {e!r}")
        return False


def kernel(x, Wq, Wkv, Wo, bo, rel_emb, _trace=False):
    import concourse.bass_utils as bu
    from concourse.bass_utils import run_bass_kernel_spmd

    if "nc" not in _CACHE:
        _CACHE["nc"] = _build_bass()
    nc = _CACHE["nc"]

    in_maps = host_prep(x, Wq, Wkv, Wo, bo, rel_emb)
    kw = {}
    if _trace and _install_ntff_hook():
        bu.upload_artifacts = lambda d: d     # zero-egress: keep artifacts local
        tmpdir = "/root/problem/traces/latest"
        import shutil
        shutil.rmtree(tmpdir, ignore_errors=True)
        os.makedirs(tmpdir, exist_ok=True)
        kw = dict(trace=True, tmpdir=tmpdir)
    res = run_bass_kernel_spmd(nc, in_maps, list(range(NCORES)), **kw)
    _CACHE["last_result"] = res

    bo = np.asarray(bo, np.float32)
    out = np.empty((B, N, DIM), np.float32)
    for b in range(B):
        pT = res.results[2 * b]["outT"] + res.results[2 * b + 1]["outT"]
        out[b] = pT.T + bo[None, :]
    return out
